# revision 11
# baseline (speedup 1.0000x reference)
"""
Trainium2 Bass kernel for windowed multi-head attention with relative position
bias (Swin-style), data-parallel over the 16 windows across 8 NeuronCores.

Reference computation (per window b of 16, N=1024 tokens, C=256 channels,
H=8 heads, hd=32):
    qkv  = x @ qkv_w.T                    -> q, k, v  [B, H, N, hd]
    attn = softmax(q k^T * hd^-0.5 + bias_table[rel_pos_index])
    out  = (attn @ v)  reshaped -> [B, N, C]
    y    = out @ proj_w.T + proj_b

Device strategy (per core: 2 windows, everything in "transposed" space):
  - host: transpose x -> xT [c, n], pre-scale q-weights by hd^-0.5; the
    gathered bias ships as a 2-byte slab whose per-slot content depends on
    the slot's exp engine (see below).
  - qT/kT co-tiles via wqkvT.T @ xT (head h at partitions [32h, 32h+32));
    v computed directly in [token, dim] layout via xT.T @ wvT.
  - The exp(score+bias) work (16.8M elems/core) is the throughput wall;
    it is split across TWO engines via a static per-slot type table:
      type M: ACT exp(scores) -> es bf16; DVE bf16-2x mul by exp(bias)
      type D: one DVE scalar_tensor_tensor: (scores*A) + slab_i16 ->
              int16 = Schraudolph bf16-bit-space exp(scores+bias)
              (A = 2^7/ln2, slab = rint(127*128 + c + A*bias), c=-7.5)
    The D fraction trades ~1.8% RMS attention-weight error (~1% total
    end-to-end) for moving exp work off the saturated ACT engine.
  - attn@v and the softmax denominator via col-tiled matmuls (4 heads /
    4 ones-stationaries whose M=32 broadcasts each colsum across its
    32-partition group), accumulating over m in PSUM (double-buffered so
    block boundaries don't stall the PE).
  - Step order (hg, nc2, b, mt): each slab is live for one contiguous
    32-slot window (ring of 2), and each output quarter (b, nc2)
    completes as soon as its two hg blocks normalize -> projection is
    emitted in quarters mid-stream, shrinking the serial tail.
  - normalize with reciprocal_approx_fast; projection computed transposed
    (yT = pwT.T @ out_catT) and untransposed on host.
"""

import functools

import ml_dtypes
import numpy as np

import concourse.bacc as bacc
import concourse.tile as tile
from concourse import mybir
from concourse.bass_utils import run_bass_kernel_spmd

BF = mybir.dt.bfloat16
F32 = mybir.dt.float32
I16 = mybir.dt.int16
NPBF = ml_dtypes.bfloat16

NCORES = 8
B = 16  # windows total
BPC = B // NCORES  # windows per core (2)
N = 1024  # tokens per window
C = 256  # channels
H = 8  # heads
HD = 32  # head dim
SCALE = HD**-0.5
NT = BPC * N  # tokens per core (2048)
EXPF = mybir.ActivationFunctionType.Exp

# Schraudolph exp in bf16-bit-space: bits = rint(x*SCH_A + 127*128 + SCH_C)
SCH_A = 2.0**7 / np.log(2.0)
SCH_C = -7.5

# D-type slots per 16 (mt, g) positions of each (nc2, hg) (b-pairs share
# slab content, so the table is b-independent). 16 -> all-D, 0 -> all-M.
ND_PER16 = 3


# D positions within each 16-slot block: keep away from the block's first
# slots (whose psc-ring stall would starve ACT behind the previous block's
# normalize in the DVE FIFO) and from the normalize slot (15).
D_PICKS = {3: (3, 7, 12), 4: (3, 6, 9, 12), 5: (2, 5, 8, 11, 13),
           6: (2, 4, 6, 9, 11, 13), 2: (5, 11), 1: (8,), 0: ()}


def _type_table():
    picks = set(D_PICKS[ND_PER16])
    tbl = {}
    for nc2 in range(2):
        for hg in range(2):
            for mt in range(8):
                for g in range(2):
                    tbl[(nc2, hg, mt, g)] = (mt * 2 + g) in picks
    return tbl


TYPE_D = _type_table()


def _emit(tc):
    nc = tc.nc
    xT_d = nc.dram_tensor("xT", [128, 2, NT], BF, kind="ExternalInput")
    wq_d = nc.dram_tensor("wqkvT", [128, 2, 3 * C], BF, kind="ExternalInput")
    pw_d = nc.dram_tensor("projwT", [128, 2, C], BF, kind="ExternalInput")
    pb_d = nc.dram_tensor("pbias", [2, 128, 1], F32, kind="ExternalInput")
    # 2-byte slab; content per slot type (bf16 exp(bias) or int16 schraudolph)
    eb_d = nc.dram_tensor("expb", [2, 2, 128, 8, 2048], I16, kind="ExternalInput")
    y_d = nc.dram_tensor("yT", [2, 128, NT], F32, kind="ExternalOutput")

    with (
        tc.tile_pool(name="const", bufs=1) as cp,
        tc.tile_pool(name="xp", bufs=1) as xp,
        tc.tile_pool(name="qkvp", bufs=1) as qkvp,
        tc.tile_pool(name="vp", bufs=1) as vp,
        tc.tile_pool(name="ebp", bufs=2) as ebp,
        tc.tile_pool(name="esp", bufs=6) as esp,
        tc.tile_pool(name="eap", bufs=8) as eap,
        tc.tile_pool(name="ocp", bufs=1) as ocp,
        tc.tile_pool(name="rcp", bufs=2) as rcp,
        tc.tile_pool(name="yp", bufs=3) as ysp,
    ):
        wq_sb = cp.tile([128, 2, 3 * C], BF)
        pw_sb = cp.tile([128, 2, C], BF)
        pb_sb = cp.tile([128, 2], F32)
        ones128 = cp.tile([128, 32], BF)
        xT_sb = xp.tile([128, 2, NT], BF)
        # qkv co-tiles: 0,1 = q heads 0-3/4-7 ; 2,3 = k ; 4,5 = v
        qkv_sb = qkvp.tile([128, 6, NT], BF)
        # v_aug blocks: [m % 128, b*8+mt, hg*128 + hl*32 + d]
        v_sb = vp.tile([128, 16, 256], BF)
        # out_catT: [co % 128, hg, n]  (co = (4*hg+hl)*32+d, n = b*1024+t)
        oc_sb = ocp.tile([128, 2, NT], BF)

        nc.sync.dma_start(wq_sb[:], wq_d[:])
        for kc in range(2):
            nc.sync.dma_start(xT_sb[:, kc, :], xT_d[:, kc, :])
        nc.sync.dma_start(pw_sb[:], pw_d[:])
        nc.sync.dma_start(pb_sb[:], pb_d.rearrange("ct p one -> p (ct one)"))
        nc.gpsimd.memset(ones128[:], 1.0)

        # ---- phase 1: qT/kT co-tiles via wqkvT.T @ xT; v directly in
        # [m, d] layout via xT.T @ wvT (no transposes needed). First slots
        # (hg=0, nc2=0, b=0) need only ct 0, 2 / nch2=0: emit those first
        # so the exp stream starts ASAP.
        with (
            tc.tile_pool(name="p1", bufs=2, space="PSUM") as p1,
            tc.tile_pool(name="pv", bufs=4, space="PSUM") as pv,
        ):
            def qk_tile(ct, nch2, eng):
                pq = p1.tile([128, 1024], F32, tag="p1", name=f"pq{ct}{nch2}")
                for half in range(2):
                    for kc in range(2):
                        nc.tensor.matmul(
                            pq[:, half * 512 : (half + 1) * 512],
                            wq_sb[:, kc, ct * 128 : (ct + 1) * 128],
                            xT_sb[
                                :,
                                kc,
                                nch2 * 1024
                                + half * 512 : nch2 * 1024
                                + (half + 1) * 512,
                            ],
                            start=(kc == 0),
                            stop=(kc == 1),
                        )
                if eng == 0:
                    nc.vector.tensor_copy(
                        qkv_sb[:, ct, nch2 * 1024 : (nch2 + 1) * 1024], pq[:]
                    )
                else:
                    nc.scalar.copy(
                        qkv_sb[:, ct, nch2 * 1024 : (nch2 + 1) * 1024], pq[:]
                    )

            def v_tile(i, eng):
                pvt = pv.tile([128, 256], F32, tag="pv", name=f"pv{i}")
                for kc in range(2):
                    nc.tensor.matmul(
                        pvt[:],
                        xT_sb[:, kc, i * 128 : (i + 1) * 128],
                        wq_sb[:, kc, 2 * C : 3 * C],
                        start=(kc == 0),
                        stop=(kc == 1),
                    )
                if eng == 0:
                    nc.vector.tensor_copy(v_sb[:, i, :], pvt[:])
                else:
                    nc.scalar.copy(v_sb[:, i, :], pvt[:])

            qk_tile(2, 0, 0)
            qk_tile(0, 0, 1)
            for i in range(8):  # v for b=0
                v_tile(i, i % 2)
            qk_tile(2, 1, 0)
            qk_tile(0, 1, 1)
            for i in range(8, 16):  # v for b=1
                v_tile(i, i % 2)
            qk_tile(3, 0, 0)
            qk_tile(1, 0, 1)
            qk_tile(3, 1, 0)
            qk_tile(1, 1, 1)

        # ---- phase 2: attention, software-pipelined over slots ----
        # A slot = one psc tile [128, 1024] covering hl-pair g of step
        # (hg, nc2, b, mt). head(slot) = scores matmuls + exp (ACT for M /
        # DVE STT for D); tail(slot) = (M: DVE bias-mul), av + cs matmuls;
        # at block end: normalize; proj per (b, nc2) quarter right after
        # both hg blocks of that quarter normalize.
        with (
            tc.tile_pool(name="psc", bufs=3, space="PSUM") as psc,
            tc.tile_pool(name="pav", bufs=1, space="PSUM") as pav,
            tc.tile_pool(name="pcs", bufs=1, space="PSUM") as pcs,
        ):
            steps = [
                (nc2, hg, mt, b)
                for hg in range(2)
                for nc2 in range(2)
                for b in range(2)
                for mt in range(8)
            ]
            slots = [(s, g) for s in steps for g in range(2)]
            blocks = {}  # (nc2, hg, b) -> dict(cs, av, rc, slab)
            state = {}  # slot -> ea-compatible AP [128, 1024] (bf16 view)

            def emit_head(slot):
                (nc2, hg, mt, b), g = slot
                if (nc2, hg, b) not in blocks:
                    cs = pcs.tile([128, 512], F32, tag="cs", name=f"cs{nc2}{hg}{b}")
                    av = pav.tile([128, 512], F32, tag="av", name=f"av{nc2}{hg}{b}")
                    rc = rcp.tile([128, 512], F32, tag="rc", name=f"rc{nc2}{hg}{b}")
                    blocks[(nc2, hg, b)] = dict(
                        slab=slabs[(nc2, hg)], cs=cs, av=av, rc=rc
                    )
                is_d = TYPE_D[(nc2, hg, mt, g)]
                scp = psc.tile([128, 1024], F32, tag="sc", name=f"sc{g}")
                for j in range(2):
                    hl = 2 * g + j
                    nc.tensor.matmul(
                        scp[:, j * 512 : (j + 1) * 512],
                        qkv_sb[
                            32 * hl : 32 * hl + 32,
                            2 + hg,
                            b * N + mt * 128 : b * N + mt * 128 + 128,
                        ],
                        qkv_sb[
                            32 * hl : 32 * hl + 32,
                            hg,
                            b * N + nc2 * 512 : b * N + nc2 * 512 + 512,
                        ],
                        start=True,
                        stop=True,
                        tile_position=(32 * hl, 0),
                    )
                slab = slabs[(nc2, hg)]
                if is_d:
                    ea = eap.tile([128, 1024], I16, tag="ea", name=f"ea{mt}{g}{b}")
                    nc.vector.scalar_tensor_tensor(
                        ea[:],
                        scp[:],
                        float(SCH_A),
                        slab[:, mt, g * 1024 : (g + 1) * 1024],
                        mybir.AluOpType.mult,
                        mybir.AluOpType.add,
                    )
                    state[slot] = (ea, True)
                else:
                    es = esp.tile([128, 1024], BF, tag="es", name=f"es{mt}{g}{b}")
                    nc.scalar.activation(es[:], scp[:], EXPF)
                    state[slot] = (es, False)

            def emit_tail(slot):
                (nc2, hg, mt, b), g = slot
                blk = blocks[(nc2, hg, b)]
                is_d = TYPE_D[(nc2, hg, mt, g)]
                ea, needs_cast = state.pop(slot)
                if not is_d:
                    eam = eap.tile([128, 1024], BF, tag="ea", name=f"eam{mt}{g}{b}")
                    nc.vector.tensor_mul(
                        eam[:],
                        ea[:],
                        blk["slab"][:, mt, g * 1024 : (g + 1) * 1024].bitcast(BF),
                    )
                    ea, needs_cast = eam, False
                for j in range(2):
                    hl = 2 * g + j
                    rhs = ea[:, j * 512 : (j + 1) * 512]
                    if needs_cast:
                        rhs = rhs.bitcast(BF)
                    nc.tensor.matmul(
                        blk["av"][32 * hl : 32 * hl + 32, :],
                        v_sb[:, b * 8 + mt, hg * 128 + 32 * hl : hg * 128 + 32 * hl + 32],
                        rhs,
                        start=(mt == 0),
                        stop=(mt == 7),
                        tile_position=(0, 32 * hl),
                        skip_group_check=True,
                    )
                    nc.tensor.matmul(
                        blk["cs"][32 * hl : 32 * hl + 32, :],
                        ones128[:],
                        rhs,
                        start=(mt == 0),
                        stop=(mt == 7),
                        tile_position=(0, 32 * hl),
                        skip_group_check=True,
                    )
                if mt == 7 and g == 1:
                    # this block's colsum is complete: reciprocal + normalize
                    nc.vector.reciprocal_approx_fast(out=blk["rc"][:], in_=blk["cs"][:])
                    nc.vector.tensor_mul(
                        oc_sb[:, hg, b * N + nc2 * 512 : b * N + nc2 * 512 + 512],
                        blk["av"][:],
                        blk["rc"][:],
                    )

            slabs = {}

            def prefetch_slab(bi):
                # first-use order of (nc2, hg) under step order (hg, nc2, b)
                nc2, hg = [(0, 0), (1, 0), (0, 1), (1, 1)][bi]
                slab = ebp.tile([128, 8, 2048], I16, tag="slab", name=f"slab{nc2}{hg}")
                nc.sync.dma_start(slab[:], eb_d[hg, nc2])
                slabs[(nc2, hg)] = slab

            def emit_proj(b, nc2):
                # yT quarter [2ct x 128, 512] for (window b, half nc2):
                # borrow one psc ring slot; contract oc over both hg chunks.
                yps = psc.tile([128, 1024], F32, tag="sc", name=f"yp{b}{nc2}")
                for ct in range(2):
                    for hg in range(2):
                        nc.tensor.matmul(
                            yps[:, ct * 512 : (ct + 1) * 512],
                            pw_sb[:, hg, ct * 128 : (ct + 1) * 128],
                            oc_sb[:, hg, b * N + nc2 * 512 : b * N + nc2 * 512 + 512],
                            start=(hg == 0),
                            stop=(hg == 1),
                            skip_group_check=True,
                        )
                yt = ysp.tile([128, 1024], F32, tag="yt", name=f"yt{b}{nc2}")
                for ct in range(2):
                    nc.vector.tensor_scalar_add(
                        yt[:, ct * 512 : (ct + 1) * 512],
                        yps[:, ct * 512 : (ct + 1) * 512],
                        pb_sb[:, ct : ct + 1],
                    )
                    nc.sync.dma_start(
                        y_d[ct, :, b * N + nc2 * 512 : b * N + nc2 * 512 + 512],
                        yt[:, ct * 512 : (ct + 1) * 512],
                    )

            SKEW = 4  # in slots
            # proj quarter (b, nc2) ready after the tails of both its hg
            # blocks; block (nc2, hg, b) ends at the slot with mt=7, g=1.
            proj_at = {}
            for b_ in range(2):
                for nc2_ in range(2):
                    last = max(
                        i
                        for i, ((nc2, hg, mt, bb), g) in enumerate(slots)
                        if nc2 == nc2_ and bb == b_ and mt == 7 and g == 1
                    )
                    proj_at.setdefault(last + SKEW + 1, []).append((b_, nc2_))

            prefetch_slab(0)
            n_slots = len(slots)
            emitted = set()
            for i, slot in enumerate(slots):
                if i % 32 == 6 and i // 32 + 1 < 4:
                    prefetch_slab(i // 32 + 1)
                emit_head(slot)
                if i >= SKEW:
                    emit_tail(slots[i - SKEW])
                for (b_, nc2_) in proj_at.get(i, ()):
                    emit_proj(b_, nc2_)
                    emitted.add((b_, nc2_))
            for j in range(SKEW, 0, -1):
                emit_tail(slots[n_slots - j])
            for b_ in range(2):
                for nc2_ in range(2):
                    if (b_, nc2_) not in emitted:
                        emit_proj(b_, nc2_)


@functools.cache
def _build_nc():
    nc = bacc.Bacc("TRN2", target_bir_lowering=False, debug=False)
    with tile.TileContext(nc) as tc:
        _emit(tc)
    nc.compile()
    return nc


def _prep_shared(qkv_w, proj_w, proj_b, bias_table, rel_pos_index):
    w2 = np.asarray(qkv_w, np.float32).copy()
    w2[:C] *= SCALE  # fold the attention scale into the q projection
    wqkvT = np.ascontiguousarray(
        w2.T.reshape(2, 128, 3 * C).transpose(1, 0, 2)
    ).astype(NPBF)
    pwT = np.ascontiguousarray(
        np.asarray(proj_w, np.float32).T.reshape(2, 128, C).transpose(1, 0, 2)
    ).astype(NPBF)
    pb = np.ascontiguousarray(
        np.asarray(proj_b, np.float32).reshape(2, 128, 1)
    )
    Braw = np.asarray(bias_table, np.float32)[np.asarray(rel_pos_index)]  # [n,m,8]
    # slab[hg, nc2, mp, mt, hl, nn], hl = 2g+j, from
    #   b = Braw[nc2*512+nn, mt*128+mp, 4hg+hl]
    # M slots: bf16(exp(b)) bits; D slots: int16(rint(A*b + 127*128 + c))
    Br = Braw.reshape(2, 512, 8, 128, 2, 4)  # [nc2, nn, mt, mp, hg, hl]
    Bt = Br.transpose(4, 0, 3, 2, 5, 1)  # [hg, nc2, mp, mt, hl, nn]
    expb = np.empty((2, 2, 128, 8, 4, 512), np.int16)
    Bc = 127 * 2**7 + SCH_C
    for nc2 in range(2):
        for hg in range(2):
            for mt in range(8):
                for hl in range(4):
                    g = hl // 2
                    chunk = Bt[hg, nc2, :, mt, hl, :]
                    if TYPE_D[(nc2, hg, mt, g)]:
                        v = np.rint(SCH_A * chunk + Bc)
                        expb[hg, nc2, :, mt, hl, :] = np.clip(
                            v, -32768, 32767
                        ).astype(np.int16)
                    else:
                        expb[hg, nc2, :, mt, hl, :] = (
                            np.exp(chunk).astype(NPBF).view(np.int16)
                        )
    expb = np.ascontiguousarray(expb.reshape(2, 2, 128, 8, 2048))
    return wqkvT, pwT, pb, expb


def _prep_x(x, c):
    xs = np.asarray(x[c * BPC : (c + 1) * BPC], np.float32)  # [2, 1024, 256]
    a = xs.transpose(2, 0, 1).reshape(C, NT)  # [c_in, b*1024+t]
    return np.ascontiguousarray(a.reshape(2, 128, NT).transpose(1, 0, 2)).astype(NPBF)


def _in_maps(x, qkv_w, proj_w, proj_b, bias_table, rel_pos_index):
    wqkvT, pwT, pb, expb = _prep_shared(
        qkv_w, proj_w, proj_b, bias_table, rel_pos_index
    )
    return [
        {
            "xT": _prep_x(x, c),
            "wqkvT": wqkvT,
            "projwT": pwT,
            "pbias": pb,
            "expb": expb,
        }
        for c in range(NCORES)
    ]


def run(x, qkv_w, proj_w, proj_b, bias_table, rel_pos_index, **run_kwargs):
    nc = _build_nc()
    in_maps = _in_maps(x, qkv_w, proj_w, proj_b, bias_table, rel_pos_index)
    res = run_bass_kernel_spmd(nc, in_maps, list(range(NCORES)), **run_kwargs)
    y = np.stack(
        [
            res.results[c]["yT"].reshape(C, NT).reshape(C, BPC, N).transpose(1, 2, 0)
            for c in range(NCORES)
        ]
    )
    return y.reshape(B, N, C).astype(np.float32), res


def kernel(x, qkv_w, proj_w, proj_b, bias_table, rel_pos_index):
    y, _ = run(x, qkv_w, proj_w, proj_b, bias_table, rel_pos_index)
    return y


# revision 12
# speedup vs baseline: 1.0463x; 1.0463x over previous
"""
Trainium2 Bass kernel for windowed multi-head attention with relative position
bias (Swin-style), data-parallel over the 16 windows across 8 NeuronCores.

Reference computation (per window b of 16, N=1024 tokens, C=256 channels,
H=8 heads, hd=32):
    qkv  = x @ qkv_w.T                    -> q, k, v  [B, H, N, hd]
    attn = softmax(q k^T * hd^-0.5 + bias_table[rel_pos_index])
    out  = (attn @ v)  reshaped -> [B, N, C]
    y    = out @ proj_w.T + proj_b

Device strategy (per core: 2 windows, everything in "transposed" space):
  - host: transpose x -> xT [c, n], pre-scale q-weights by hd^-0.5; the
    gathered bias ships as a 2-byte slab whose per-slot content depends on
    the slot's exp engine (see below).
  - qT/kT co-tiles via wqkvT.T @ xT (head h at partitions [32h, 32h+32));
    v computed directly in [token, dim] layout via xT.T @ wvT.
  - The exp(score+bias) work (16.8M elems/core) is the throughput wall;
    it is split across TWO engines via a static per-slot type table:
      type M: ACT exp(scores) -> es bf16; DVE bf16-2x mul by exp(bias)
      type D: one DVE scalar_tensor_tensor: (scores*A) + slab_i16 ->
              int16 = Schraudolph bf16-bit-space exp(scores+bias)
              (A = 2^7/ln2, slab = rint(127*128 + c + A*bias), c=-7.5)
    The D fraction trades ~1.8% RMS attention-weight error (~1% total
    end-to-end) for moving exp work off the saturated ACT engine.
  - attn@v and the softmax denominator via col-tiled matmuls (4 heads /
    4 ones-stationaries whose M=32 broadcasts each colsum across its
    32-partition group), accumulating over m in PSUM (double-buffered so
    block boundaries don't stall the PE).
  - Step order (hg, nc2, b, mt): each slab is live for one contiguous
    32-slot window (ring of 2), and each output quarter (b, nc2)
    completes as soon as its two hg blocks normalize -> projection is
    emitted in quarters mid-stream, shrinking the serial tail.
  - normalize with reciprocal_approx_fast; projection computed transposed
    (yT = pwT.T @ out_catT) and untransposed on host.
"""

import functools

import ml_dtypes
import numpy as np

import concourse.bacc as bacc
import concourse.tile as tile
from concourse import mybir
from concourse.bass_utils import run_bass_kernel_spmd

BF = mybir.dt.bfloat16
F32 = mybir.dt.float32
I16 = mybir.dt.int16
NPBF = ml_dtypes.bfloat16

NCORES = 8
B = 16  # windows total
BPC = B // NCORES  # windows per core (2)
N = 1024  # tokens per window
C = 256  # channels
H = 8  # heads
HD = 32  # head dim
SCALE = HD**-0.5
NT = BPC * N  # tokens per core (2048)
EXPF = mybir.ActivationFunctionType.Exp

# Schraudolph exp in bf16-bit-space: bits = rint(x*SCH_A + 127*128 + SCH_C)
SCH_A = 2.0**7 / np.log(2.0)
SCH_C = -7.5

# D-type slots per 16 (mt, g) positions of each (nc2, hg) (b-pairs share
# slab content, so the table is b-independent). 16 -> all-D, 0 -> all-M.
ND_PER16 = 3


# D positions within each 16-slot block: keep away from the block's first
# slots (whose psc-ring stall would starve ACT behind the previous block's
# normalize in the DVE FIFO) and from the normalize slot (15).
D_PICKS = {3: (0, 5, 11), 4: (0, 4, 8, 12), 5: (0, 3, 6, 9, 12),
           6: (0, 3, 6, 9, 12, 14), 2: (5, 11), 1: (8,), 0: ()}


def _type_table():
    picks = set(D_PICKS[ND_PER16])
    tbl = {}
    for nc2 in range(2):
        for hg in range(2):
            for mt in range(8):
                for g in range(2):
                    tbl[(nc2, hg, mt, g)] = (mt * 2 + g) in picks
    return tbl


TYPE_D = _type_table()


def _emit(tc):
    nc = tc.nc
    xT_d = nc.dram_tensor("xT", [128, 2, NT], BF, kind="ExternalInput")
    wq_d = nc.dram_tensor("wqkvT", [128, 2, 3 * C], BF, kind="ExternalInput")
    pw_d = nc.dram_tensor("projwT", [128, 2, C], BF, kind="ExternalInput")
    pb_d = nc.dram_tensor("pbias", [2, 128, 1], F32, kind="ExternalInput")
    # 2-byte slab; content per slot type (bf16 exp(bias) or int16 schraudolph)
    eb_d = nc.dram_tensor("expb", [2, 2, 128, 8, 2048], I16, kind="ExternalInput")
    y_d = nc.dram_tensor("yT", [2, 128, NT], F32, kind="ExternalOutput")

    with (
        tc.tile_pool(name="const", bufs=1) as cp,
        tc.tile_pool(name="xp", bufs=1) as xp,
        tc.tile_pool(name="qkvp", bufs=1) as qkvp,
        tc.tile_pool(name="vp", bufs=1) as vp,
        tc.tile_pool(name="ebp", bufs=2) as ebp,
        tc.tile_pool(name="esp", bufs=8) as esp,
        tc.tile_pool(name="eap", bufs=10) as eap,
        tc.tile_pool(name="ocp", bufs=1) as ocp,
        tc.tile_pool(name="rcp", bufs=2) as rcp,
        tc.tile_pool(name="yp", bufs=3) as ysp,
    ):
        wq_sb = cp.tile([128, 2, 3 * C], BF)
        pw_sb = cp.tile([128, 2, C], BF)
        pb_sb = cp.tile([128, 2], F32)
        ones128 = cp.tile([128, 32], BF)
        xT_sb = xp.tile([128, 2, NT], BF)
        # qkv co-tiles: 0,1 = q heads 0-3/4-7 ; 2,3 = k ; 4,5 = v
        qkv_sb = qkvp.tile([128, 6, NT], BF)
        # v_aug blocks: [m % 128, b*8+mt, hg*128 + hl*32 + d]
        v_sb = vp.tile([128, 16, 256], BF)
        # out_catT: [co % 128, hg, n]  (co = (4*hg+hl)*32+d, n = b*1024+t)
        oc_sb = ocp.tile([128, 2, NT], BF)

        nc.sync.dma_start(wq_sb[:], wq_d[:])
        for kc in range(2):
            nc.sync.dma_start(xT_sb[:, kc, :], xT_d[:, kc, :])
        nc.sync.dma_start(pw_sb[:], pw_d[:])
        nc.sync.dma_start(pb_sb[:], pb_d.rearrange("ct p one -> p (ct one)"))
        nc.gpsimd.memset(ones128[:], 1.0)

        # ---- phase 1: qT/kT co-tiles via wqkvT.T @ xT; v directly in
        # [m, d] layout via xT.T @ wvT (no transposes needed). First slots
        # (hg=0, nc2=0, b=0) need only ct 0, 2 / nch2=0: emit those first
        # so the exp stream starts ASAP.
        with (
            tc.tile_pool(name="p1", bufs=2, space="PSUM") as p1,
            tc.tile_pool(name="pv", bufs=4, space="PSUM") as pv,
        ):
            def qk_tile(ct, nch2, eng):
                pq = p1.tile([128, 1024], F32, tag="p1", name=f"pq{ct}{nch2}")
                for half in range(2):
                    for kc in range(2):
                        nc.tensor.matmul(
                            pq[:, half * 512 : (half + 1) * 512],
                            wq_sb[:, kc, ct * 128 : (ct + 1) * 128],
                            xT_sb[
                                :,
                                kc,
                                nch2 * 1024
                                + half * 512 : nch2 * 1024
                                + (half + 1) * 512,
                            ],
                            start=(kc == 0),
                            stop=(kc == 1),
                        )
                if eng == 0:
                    nc.vector.tensor_copy(
                        qkv_sb[:, ct, nch2 * 1024 : (nch2 + 1) * 1024], pq[:]
                    )
                else:
                    nc.scalar.copy(
                        qkv_sb[:, ct, nch2 * 1024 : (nch2 + 1) * 1024], pq[:]
                    )

            def v_tile(i, eng):
                pvt = pv.tile([128, 256], F32, tag="pv", name=f"pv{i}")
                for kc in range(2):
                    nc.tensor.matmul(
                        pvt[:],
                        xT_sb[:, kc, i * 128 : (i + 1) * 128],
                        wq_sb[:, kc, 2 * C : 3 * C],
                        start=(kc == 0),
                        stop=(kc == 1),
                    )
                if eng == 0:
                    nc.vector.tensor_copy(v_sb[:, i, :], pvt[:])
                else:
                    nc.scalar.copy(v_sb[:, i, :], pvt[:])

            qk_tile(2, 0, 0)
            qk_tile(0, 0, 1)
            for i in range(8):  # v for b=0
                v_tile(i, i % 2)
            qk_tile(2, 1, 0)
            qk_tile(0, 1, 1)
            for i in range(8, 16):  # v for b=1
                v_tile(i, i % 2)
            qk_tile(3, 0, 0)
            qk_tile(1, 0, 1)
            qk_tile(3, 1, 0)
            qk_tile(1, 1, 1)

        # ---- phase 2: attention, software-pipelined over slots ----
        # A slot = one psc tile [128, 1024] covering hl-pair g of step
        # (hg, nc2, b, mt). head(slot) = scores matmuls + exp (ACT for M /
        # DVE STT for D); tail(slot) = (M: DVE bias-mul), av + cs matmuls;
        # at block end: normalize; proj per (b, nc2) quarter right after
        # both hg blocks of that quarter normalize.
        with (
            tc.tile_pool(name="psc", bufs=2, space="PSUM") as psc,
            tc.tile_pool(name="psd", bufs=1, space="PSUM") as psd,
            tc.tile_pool(name="pav", bufs=1, space="PSUM") as pav,
            tc.tile_pool(name="pcs", bufs=1, space="PSUM") as pcs,
        ):
            steps = [
                (nc2, hg, mt, b)
                for hg in range(2)
                for nc2 in range(2)
                for b in range(2)
                for mt in range(8)
            ]
            slots = [(s, g) for s in steps for g in range(2)]
            blocks = {}  # (nc2, hg, b) -> dict(cs, av, rc, slab)
            state = {}  # slot -> ea-compatible AP [128, 1024] (bf16 view)

            def emit_head(slot):
                (nc2, hg, mt, b), g = slot
                if (nc2, hg, b) not in blocks:
                    cs = pcs.tile([128, 512], F32, tag="cs", name=f"cs{nc2}{hg}{b}")
                    av = pav.tile([128, 512], F32, tag="av", name=f"av{nc2}{hg}{b}")
                    rc = rcp.tile([128, 512], F32, tag="rc", name=f"rc{nc2}{hg}{b}")
                    blocks[(nc2, hg, b)] = dict(
                        slab=slabs[(nc2, hg)], cs=cs, av=av, rc=rc
                    )
                is_d = TYPE_D[(nc2, hg, mt, g)]
                if is_d:
                    scp = psd.tile([128, 1024], F32, tag="sd", name=f"sd{g}")
                else:
                    scp = psc.tile([128, 1024], F32, tag="sc", name=f"sc{g}")
                for j in range(2):
                    hl = 2 * g + j
                    nc.tensor.matmul(
                        scp[:, j * 512 : (j + 1) * 512],
                        qkv_sb[
                            32 * hl : 32 * hl + 32,
                            2 + hg,
                            b * N + mt * 128 : b * N + mt * 128 + 128,
                        ],
                        qkv_sb[
                            32 * hl : 32 * hl + 32,
                            hg,
                            b * N + nc2 * 512 : b * N + nc2 * 512 + 512,
                        ],
                        start=True,
                        stop=True,
                        tile_position=(32 * hl, 0),
                    )
                slab = slabs[(nc2, hg)]
                if is_d:
                    ea = eap.tile([128, 1024], I16, tag="ea", name=f"ea{mt}{g}{b}")
                    nc.vector.scalar_tensor_tensor(
                        ea[:],
                        scp[:],
                        float(SCH_A),
                        slab[:, mt, g * 1024 : (g + 1) * 1024],
                        mybir.AluOpType.mult,
                        mybir.AluOpType.add,
                    )
                    state[slot] = (ea, True)
                else:
                    es = esp.tile([128, 1024], BF, tag="es", name=f"es{mt}{g}{b}")
                    nc.scalar.activation(es[:], scp[:], EXPF)
                    state[slot] = (es, False)

            def emit_tail(slot):
                (nc2, hg, mt, b), g = slot
                blk = blocks[(nc2, hg, b)]
                is_d = TYPE_D[(nc2, hg, mt, g)]
                ea, needs_cast = state.pop(slot)
                if not is_d:
                    eam = eap.tile([128, 1024], BF, tag="ea", name=f"eam{mt}{g}{b}")
                    nc.vector.tensor_mul(
                        eam[:],
                        ea[:],
                        blk["slab"][:, mt, g * 1024 : (g + 1) * 1024].bitcast(BF),
                    )
                    ea, needs_cast = eam, False
                for j in range(2):
                    hl = 2 * g + j
                    rhs = ea[:, j * 512 : (j + 1) * 512]
                    if needs_cast:
                        rhs = rhs.bitcast(BF)
                    nc.tensor.matmul(
                        blk["av"][32 * hl : 32 * hl + 32, :],
                        v_sb[:, b * 8 + mt, hg * 128 + 32 * hl : hg * 128 + 32 * hl + 32],
                        rhs,
                        start=(mt == 0),
                        stop=(mt == 7),
                        tile_position=(0, 32 * hl),
                        skip_group_check=True,
                    )
                    nc.tensor.matmul(
                        blk["cs"][32 * hl : 32 * hl + 32, :],
                        ones128[:],
                        rhs,
                        start=(mt == 0),
                        stop=(mt == 7),
                        tile_position=(0, 32 * hl),
                        skip_group_check=True,
                    )
                if mt == 7 and g == 1:
                    # this block's colsum is complete: reciprocal + normalize
                    nc.vector.reciprocal_approx_fast(out=blk["rc"][:], in_=blk["cs"][:])
                    nc.vector.tensor_mul(
                        oc_sb[:, hg, b * N + nc2 * 512 : b * N + nc2 * 512 + 512],
                        blk["av"][:],
                        blk["rc"][:],
                    )

            slabs = {}

            def prefetch_slab(bi):
                # first-use order of (nc2, hg) under step order (hg, nc2, b)
                nc2, hg = [(0, 0), (1, 0), (0, 1), (1, 1)][bi]
                slab = ebp.tile([128, 8, 2048], I16, tag="slab", name=f"slab{nc2}{hg}")
                nc.sync.dma_start(slab[:], eb_d[hg, nc2])
                slabs[(nc2, hg)] = slab

            def emit_proj(b, nc2):
                # yT quarter [2ct x 128, 512] for (window b, half nc2):
                # borrow one psc ring slot; contract oc over both hg chunks.
                yps = psc.tile([128, 1024], F32, tag="sc", name=f"yp{b}{nc2}")
                for ct in range(2):
                    for hg in range(2):
                        nc.tensor.matmul(
                            yps[:, ct * 512 : (ct + 1) * 512],
                            pw_sb[:, hg, ct * 128 : (ct + 1) * 128],
                            oc_sb[:, hg, b * N + nc2 * 512 : b * N + nc2 * 512 + 512],
                            start=(hg == 0),
                            stop=(hg == 1),
                            skip_group_check=True,
                        )
                yt = ysp.tile([128, 1024], F32, tag="yt", name=f"yt{b}{nc2}")
                for ct in range(2):
                    nc.vector.tensor_scalar_add(
                        yt[:, ct * 512 : (ct + 1) * 512],
                        yps[:, ct * 512 : (ct + 1) * 512],
                        pb_sb[:, ct : ct + 1],
                    )
                    nc.sync.dma_start(
                        y_d[ct, :, b * N + nc2 * 512 : b * N + nc2 * 512 + 512],
                        yt[:, ct * 512 : (ct + 1) * 512],
                    )

            SKEW = 4  # in slots
            # proj quarter (b, nc2) ready after the tails of both its hg
            # blocks; block (nc2, hg, b) ends at the slot with mt=7, g=1.
            proj_at = {}
            for b_ in range(2):
                for nc2_ in range(2):
                    last = max(
                        i
                        for i, ((nc2, hg, mt, bb), g) in enumerate(slots)
                        if nc2 == nc2_ and bb == b_ and mt == 7 and g == 1
                    )
                    proj_at.setdefault(last + SKEW + 1, []).append((b_, nc2_))

            prefetch_slab(0)
            n_slots = len(slots)
            emitted = set()
            for i, slot in enumerate(slots):
                if i % 32 == 6 and i // 32 + 1 < 4:
                    prefetch_slab(i // 32 + 1)
                emit_head(slot)
                if i >= SKEW:
                    emit_tail(slots[i - SKEW])
                for (b_, nc2_) in proj_at.get(i, ()):
                    emit_proj(b_, nc2_)
                    emitted.add((b_, nc2_))
            for j in range(SKEW, 0, -1):
                emit_tail(slots[n_slots - j])
            for b_ in range(2):
                for nc2_ in range(2):
                    if (b_, nc2_) not in emitted:
                        emit_proj(b_, nc2_)


@functools.cache
def _build_nc():
    nc = bacc.Bacc("TRN2", target_bir_lowering=False, debug=False)
    with tile.TileContext(nc) as tc:
        _emit(tc)
    nc.compile()
    return nc


def _prep_shared(qkv_w, proj_w, proj_b, bias_table, rel_pos_index):
    w2 = np.asarray(qkv_w, np.float32).copy()
    w2[:C] *= SCALE  # fold the attention scale into the q projection
    wqkvT = np.ascontiguousarray(
        w2.T.reshape(2, 128, 3 * C).transpose(1, 0, 2)
    ).astype(NPBF)
    pwT = np.ascontiguousarray(
        np.asarray(proj_w, np.float32).T.reshape(2, 128, C).transpose(1, 0, 2)
    ).astype(NPBF)
    pb = np.ascontiguousarray(
        np.asarray(proj_b, np.float32).reshape(2, 128, 1)
    )
    Braw = np.asarray(bias_table, np.float32)[np.asarray(rel_pos_index)]  # [n,m,8]
    # slab[hg, nc2, mp, mt, hl, nn], hl = 2g+j, from
    #   b = Braw[nc2*512+nn, mt*128+mp, 4hg+hl]
    # M slots: bf16(exp(b)) bits; D slots: int16(rint(A*b + 127*128 + c))
    Br = Braw.reshape(2, 512, 8, 128, 2, 4)  # [nc2, nn, mt, mp, hg, hl]
    Bt = Br.transpose(4, 0, 3, 2, 5, 1)  # [hg, nc2, mp, mt, hl, nn]
    expb = np.empty((2, 2, 128, 8, 4, 512), np.int16)
    Bc = 127 * 2**7 + SCH_C
    for nc2 in range(2):
        for hg in range(2):
            for mt in range(8):
                for hl in range(4):
                    g = hl // 2
                    chunk = Bt[hg, nc2, :, mt, hl, :]
                    if TYPE_D[(nc2, hg, mt, g)]:
                        v = np.rint(SCH_A * chunk + Bc)
                        expb[hg, nc2, :, mt, hl, :] = np.clip(
                            v, -32768, 32767
                        ).astype(np.int16)
                    else:
                        expb[hg, nc2, :, mt, hl, :] = (
                            np.exp(chunk).astype(NPBF).view(np.int16)
                        )
    expb = np.ascontiguousarray(expb.reshape(2, 2, 128, 8, 2048))
    return wqkvT, pwT, pb, expb


def _prep_x(x, c):
    xs = np.asarray(x[c * BPC : (c + 1) * BPC], np.float32)  # [2, 1024, 256]
    a = xs.transpose(2, 0, 1).reshape(C, NT)  # [c_in, b*1024+t]
    return np.ascontiguousarray(a.reshape(2, 128, NT).transpose(1, 0, 2)).astype(NPBF)


def _in_maps(x, qkv_w, proj_w, proj_b, bias_table, rel_pos_index):
    wqkvT, pwT, pb, expb = _prep_shared(
        qkv_w, proj_w, proj_b, bias_table, rel_pos_index
    )
    return [
        {
            "xT": _prep_x(x, c),
            "wqkvT": wqkvT,
            "projwT": pwT,
            "pbias": pb,
            "expb": expb,
        }
        for c in range(NCORES)
    ]


def run(x, qkv_w, proj_w, proj_b, bias_table, rel_pos_index, **run_kwargs):
    nc = _build_nc()
    in_maps = _in_maps(x, qkv_w, proj_w, proj_b, bias_table, rel_pos_index)
    res = run_bass_kernel_spmd(nc, in_maps, list(range(NCORES)), **run_kwargs)
    y = np.stack(
        [
            res.results[c]["yT"].reshape(C, NT).reshape(C, BPC, N).transpose(1, 2, 0)
            for c in range(NCORES)
        ]
    )
    return y.reshape(B, N, C).astype(np.float32), res


def kernel(x, qkv_w, proj_w, proj_b, bias_table, rel_pos_index):
    y, _ = run(x, qkv_w, proj_w, proj_b, bias_table, rel_pos_index)
    return y


# revision 22
# speedup vs baseline: 1.1138x; 1.0646x over previous
"""
Trainium2 Bass kernel for windowed multi-head attention with relative position
bias (Swin-style), data-parallel over the 16 windows across 8 NeuronCores.

Reference computation (per window b of 16, N=1024 tokens, C=256 channels,
H=8 heads, hd=32):
    qkv  = x @ qkv_w.T                    -> q, k, v  [B, H, N, hd]
    attn = softmax(q k^T * hd^-0.5 + bias_table[rel_pos_index])
    out  = (attn @ v)  reshaped -> [B, N, C]
    y    = out @ proj_w.T + proj_b

Device strategy (per core: 2 windows, everything in "transposed" space):
  - host: transpose x -> xT [c, n], pre-scale q-weights by hd^-0.5, gather
    exp(bias_table)[rel_pos_index] to bf16 (softmax is factorized as
    exp(s)*exp(bias), valid since scores are bounded ~|s|<12).
  - qT/kT co-tiles via wqkvT.T @ xT (head h at partitions [32h, 32h+32),
    the layout needed for 4-way tile_position packing of K=32 score
    matmuls); v computed directly in [token, dim] layout via xT.T @ wvT
    (no on-chip transposes anywhere).
  - scoresT[m,n] via row-tiled matmuls; ACT exp(scoresT) PSUM->SBUF bf16
    is the throughput wall (16.8M exps/core, ~1.1us per FD-1024 call);
    DVE does one bf16 2x multiply by the gathered exp-bias per tile.
  - attn@v and the softmax denominator via col-tiled matmuls (4 heads /
    4 ones-stationaries whose M=32 broadcasts each colsum across its
    32-partition group), accumulating over m in PSUM.
  - Software-pipelined emission (scores+exp of step i before mul/av/cs of
    step i-2) keeps the per-engine FIFOs from putting accumulation work in
    the exp ring; PSUM: 3-slot score ring (6 banks) + av + cs (1 each).
  - normalize with reciprocal_approx_fast; projection computed transposed
    (yT = pwT.T @ out_catT, N=512 matmuls) and untransposed on host.
"""

import functools

import ml_dtypes
import numpy as np

import concourse.bacc as bacc
import concourse.tile as tile
from concourse import mybir
from concourse.bass_utils import run_bass_kernel_spmd

BF = mybir.dt.bfloat16
F32 = mybir.dt.float32
NPBF = ml_dtypes.bfloat16

NCORES = 8
B = 16  # windows total
BPC = B // NCORES  # windows per core (2)
N = 1024  # tokens per window
C = 256  # channels
H = 8  # heads
HD = 32  # head dim
SCALE = HD**-0.5
NT = BPC * N  # tokens per core (2048)
EXPF = mybir.ActivationFunctionType.Exp
I16 = mybir.dt.int16

import os
# Schraudolph exp in bf16-bit-space: bits = rint(x*SCH_A + 127*128 + SCH_C)
SCH_A = 2.0**7 / np.log(2.0)
SCH_C = -7.5
K_ND = 0  # D-steps of the 32 (nc2,hg,mt); 0 = all exp on ScalarE

def _type_table():
    # spread K_ND D-steps over the 32 (nc2, hg, mt) positions, mid-block
    # (avoid mt 0 and 7); b windows share slab content -> b-independent.
    slots = [(nc2, hg, mt) for nc2 in range(2) for hg in range(2)
             for mt in (3, 5, 1, 6, 2, 4, 0, 7)]
    order = []
    for mtrank in range(8):
        for q in range(4):
            order.append(slots[q * 8 + mtrank])
    picks = set(order[:K_ND])
    return {s: (s in picks) for s in
            [(nc2, hg, mt) for nc2 in range(2) for hg in range(2)
             for mt in range(8)]}

TYPE_D = _type_table()


def _emit(tc):
    nc = tc.nc
    xT_d = nc.dram_tensor("xT", [128, 2, NT], BF, kind="ExternalInput")
    wq_d = nc.dram_tensor("wqkvT", [128, 2, 3 * C], BF, kind="ExternalInput")
    pw_d = nc.dram_tensor("projwT", [128, 2, C], BF, kind="ExternalInput")
    pb_d = nc.dram_tensor("pbias", [2, 128, 1], F32, kind="ExternalInput")
    eb_d = nc.dram_tensor("expb", [2, 2, 128, 8, 2048], I16, kind="ExternalInput")
    y_d = nc.dram_tensor("yT", [2, 128, NT], F32, kind="ExternalOutput")

    with (
        tc.tile_pool(name="const", bufs=1) as cp,
        tc.tile_pool(name="xp", bufs=1) as xp,
        tc.tile_pool(name="qkvp", bufs=1) as qkvp,
        tc.tile_pool(name="vp", bufs=1) as vp,
        tc.tile_pool(name="ebp", bufs=2) as ebp,
        tc.tile_pool(name="esp", bufs=6) as esp,
        tc.tile_pool(name="eap", bufs=4) as eap,
        tc.tile_pool(name="ocp", bufs=1) as ocp,
        tc.tile_pool(name="rcp", bufs=2) as rcp,
        tc.tile_pool(name="yp", bufs=3) as ysp,
    ):
        wq_sb = cp.tile([128, 2, 3 * C], BF)
        pw_sb = cp.tile([128, 2, C], BF)
        pb_sb = cp.tile([128, 2], F32)
        ones128 = cp.tile([128, 32], BF)
        xT_sb = xp.tile([128, 2, NT], BF)
        # qkv co-tiles: 0,1 = q heads 0-3/4-7 ; 2,3 = k ; 4,5 = v
        qkv_sb = qkvp.tile([128, 6, NT], BF)
        # v_aug blocks: [m % 128, b*8+mt, hg*128 + hl*32 + d]
        v_sb = vp.tile([128, 16, 256], BF)
        # out_catT: [co % 128, hg, n]  (co = (4*hg+hl)*32+d, n = b*1024+t)
        oc_sb = ocp.tile([128, 2, NT], BF)

        nc.sync.dma_start(wq_sb[:], wq_d[:])
        if False:  # xT DMA chunking: measured neutral
            for kc in range(2):
                for h in range(2):
                    nc.sync.dma_start(
                        xT_sb[:, kc, h * 1024 : (h + 1) * 1024],
                        xT_d[:, kc, h * 1024 : (h + 1) * 1024],
                    )
        else:
            for kc in range(2):
                nc.sync.dma_start(xT_sb[:, kc, :], xT_d[:, kc, :])
        nc.sync.dma_start(pw_sb[:], pw_d[:])
        nc.sync.dma_start(pb_sb[:], pb_d.rearrange("ct p one -> p (ct one)"))
        nc.gpsimd.memset(ones128[:], 1.0)

        # ---- phase 1: qT/kT co-tiles via wqkvT.T @ xT; v directly in
        # [m, d] layout via xT.T @ wvT (no transposes needed). Evacuations
        # alternate DVE / ScalarE (ScalarE is idle before the exp stream).
        with (
            tc.tile_pool(name="p1", bufs=2, space="PSUM") as p1,
            tc.tile_pool(name="pv", bufs=4, space="PSUM") as pv,
        ):
            def qk_tile(ct, nch2, eng):
                pq = p1.tile([128, 1024], F32, tag="p1", name=f"pq{ct}{nch2}")
                for half in range(2):
                    for kc in range(2):
                        nc.tensor.matmul(
                            pq[:, half * 512 : (half + 1) * 512],
                            wq_sb[:, kc, ct * 128 : (ct + 1) * 128],
                            xT_sb[
                                :,
                                kc,
                                nch2 * 1024
                                + half * 512 : nch2 * 1024
                                + (half + 1) * 512,
                            ],
                            start=(kc == 0),
                            stop=(kc == 1),
                        )
                if eng == 0:
                    nc.vector.tensor_copy(
                        qkv_sb[:, ct, nch2 * 1024 : (nch2 + 1) * 1024], pq[:]
                    )
                else:
                    nc.scalar.copy(
                        qkv_sb[:, ct, nch2 * 1024 : (nch2 + 1) * 1024], pq[:]
                    )

            def v_tile(i, eng):
                pvt = pv.tile([128, 256], F32, tag="pv", name=f"pv{i}")
                for kc in range(2):
                    nc.tensor.matmul(
                        pvt[:],
                        xT_sb[:, kc, i * 128 : (i + 1) * 128],
                        wq_sb[:, kc, 2 * C : 3 * C],
                        start=(kc == 0),
                        stop=(kc == 1),
                    )
                if eng == 0:
                    nc.vector.tensor_copy(v_sb[:, i, :], pvt[:])
                else:
                    nc.scalar.copy(v_sb[:, i, :], pvt[:])

            K_EVAC = "alt"
            def _eng(i):
                return (i % 2) if K_EVAC == "alt" else 0
            e = 0
            if False:  # early qk order: measured neutral
                for ct, nch2 in ((2, 0), (0, 0), (2, 1), (0, 1)):
                    qk_tile(ct, nch2, _eng(e)); e += 1
                for i in range(16):
                    v_tile(i, _eng(i))
                for ct, nch2 in ((3, 0), (1, 0), (3, 1), (1, 1)):
                    qk_tile(ct, nch2, _eng(e)); e += 1
            else:
                for ct in (0, 2):
                    for nch2 in range(2):
                        qk_tile(ct, nch2, _eng(e))
                        e += 1
                for i in range(16):
                    v_tile(i, _eng(i))
                for ct in (1, 3):
                    for nch2 in range(2):
                        qk_tile(ct, nch2, _eng(e))
                        e += 1

        # ---- phase 2: attention, software-pipelined ----
        # Per step (nc2, hg, mt, b): emit the scores matmuls + exps FIRST, then
        # the previous step's bias-mul / attn@v / colsum. This keeps next-step
        # scores ahead of av/cs in the PE FIFO so the ACT engine's ring
        # (exp -> scores -> exp) never includes the accumulation matmuls.
        with (
            tc.tile_pool(name="psc", bufs=3, space="PSUM") as psc,
            tc.tile_pool(name="pav", bufs=1, space="PSUM") as pav,
            tc.tile_pool(name="pcs", bufs=1, space="PSUM") as pcs,
        ):
            steps = [
                (nc2, hg, mt, b)
                for nc2 in range(2)
                for hg in range(2)
                for b in range(2)
                for mt in range(8)
            ]
            blocks = {}  # (nc2, hg) -> dict(cs, avs, rc, slab)
            state = {}  # step -> (es, ea)

            def emit_head(step):
                nc2, hg, mt, b = step
                if (nc2, hg, b) not in blocks:
                    cs = pcs.tile([128, 512], F32, tag="cs", name=f"cs{nc2}{hg}{b}")
                    av = pav.tile([128, 512], F32, tag="av", name=f"av{nc2}{hg}{b}")
                    rc = rcp.tile([128, 512], F32, tag="rc", name=f"rc{nc2}{hg}{b}")
                    blocks[(nc2, hg, b)] = dict(
                        slab=slabs[(nc2, hg)], cs=cs, av=av, rc=rc
                    )
                is_d = TYPE_D[(nc2, hg, mt)]
                if is_d:
                    es = esp.tile([128, 2048], I16, tag="es", name=f"es{mt}{b}")
                else:
                    es = esp.tile([128, 2048], BF, tag="es", name=f"es{mt}{b}")
                for g in range(2):
                    scp = psc.tile([128, 1024], F32, tag="sc", name=f"sc{g}")
                    for j in range(2):
                        hl = 2 * g + j
                        nc.tensor.matmul(
                            scp[:, j * 512 : (j + 1) * 512],
                            qkv_sb[
                                32 * hl : 32 * hl + 32,
                                2 + hg,
                                b * N + mt * 128 : b * N + mt * 128 + 128,
                            ],
                            qkv_sb[
                                32 * hl : 32 * hl + 32,
                                hg,
                                b * N + nc2 * 512 : b * N + nc2 * 512 + 512,
                            ],
                            start=True,
                            stop=True,
                            tile_position=(32 * hl, 0),
                        )
                    if is_d:
                        nc.vector.scalar_tensor_tensor(
                            es[:, g * 1024 : (g + 1) * 1024],
                            scp[:],
                            float(SCH_A),
                            slabs[(nc2, hg)][:, mt, g * 1024 : (g + 1) * 1024],
                            mybir.AluOpType.mult,
                            mybir.AluOpType.add,
                        )
                    else:
                        nc.scalar.activation(
                            es[:, g * 1024 : (g + 1) * 1024], scp[:], EXPF
                        )
                state[step] = es

            def emit_tail(step):
                nc2, hg, mt, b = step
                blk = blocks[(nc2, hg, b)]
                es = state.pop(step)
                is_d = TYPE_D[(nc2, hg, mt)]
                if is_d:
                    ea = es.bitcast(BF) if hasattr(es, "bitcast") else es
                    ea_ap = es[:].bitcast(BF)
                else:
                    ea = eap.tile([128, 2048], BF)
                    nc.vector.tensor_mul(
                        ea[:], es[:], blk["slab"][:, mt, :].bitcast(BF)
                    )
                    ea_ap = ea[:]
                for hl in range(4):
                    nc.tensor.matmul(
                        blk["av"][32 * hl : 32 * hl + 32, :],
                        v_sb[:, b * 8 + mt, hg * 128 + 32 * hl : hg * 128 + 32 * hl + 32],
                        ea_ap[:, hl * 512 : (hl + 1) * 512],
                        start=(mt == 0),
                        stop=(mt == 7),
                        tile_position=(0, 32 * hl),
                        skip_group_check=True,
                    )
                    nc.tensor.matmul(
                        blk["cs"][32 * hl : 32 * hl + 32, :],
                        ones128[:],
                        ea_ap[:, hl * 512 : (hl + 1) * 512],
                        start=(mt == 0),
                        stop=(mt == 7),
                        tile_position=(0, 32 * hl),
                        skip_group_check=True,
                    )
                if mt == 7:
                    # this window's colsum is complete: reciprocal + normalize
                    nc.vector.reciprocal_approx_fast(out=blk["rc"][:], in_=blk["cs"][:])
                    nc.vector.tensor_mul(
                        oc_sb[:, hg, b * N + nc2 * 512 : b * N + nc2 * 512 + 512],
                        blk["av"][:],
                        blk["rc"][:],
                    )

            slabs = {}

            def prefetch_slab(bi):
                nc2, hg = [(n, h) for n in range(2) for h in range(2)][bi]
                slab = ebp.tile([128, 8, 2048], I16, tag="slab", name=f"slab{nc2}{hg}")
                nc.sync.dma_start(slab[:], eb_d[hg, nc2])
                slabs[(nc2, hg)] = slab

            def emit_proj(ct, nch):
                yps = psc.tile([128, 1024], F32, tag="sc", name=f"yps{ct}{nch}")
                for half in range(2):
                    for hg in range(2):
                        nc.tensor.matmul(
                            yps[:, half * 512 : (half + 1) * 512],
                            pw_sb[:, hg, ct * 128 : (ct + 1) * 128],
                            oc_sb[
                                :,
                                hg,
                                nch * 1024 + half * 512 : nch * 1024 + (half + 1) * 512,
                            ],
                            start=(hg == 0),
                            stop=(hg == 1),
                            skip_group_check=True,
                        )
                yt = ysp.tile([128, 1024], F32, tag="yt", name=f"yt{ct}{nch}")
                nc.vector.tensor_scalar_add(yt[:], yps[:], pb_sb[:, ct : ct + 1])
                nc.sync.dma_start(y_d[ct, :, nch * 1024 : (nch + 1) * 1024], yt[:])

            def emit_proj_quarter(b, nc2):
                # yT quarter for (window b, half nc2): contract oc over both
                # hg chunks; borrow one sc-ring slot for both ct chunks.
                yps = psc.tile([128, 1024], F32, tag="sc", name=f"ypq{b}{nc2}")
                for ct in range(2):
                    for hg in range(2):
                        nc.tensor.matmul(
                            yps[:, ct * 512 : (ct + 1) * 512],
                            pw_sb[:, hg, ct * 128 : (ct + 1) * 128],
                            oc_sb[:, hg, b * N + nc2 * 512 : b * N + nc2 * 512 + 512],
                            start=(hg == 0),
                            stop=(hg == 1),
                            skip_group_check=True,
                        )
                yt = ysp.tile([128, 1024], F32, tag="yt", name=f"ytq{b}{nc2}")
                for ct in range(2):
                    nc.vector.tensor_scalar_add(
                        yt[:, ct * 512 : (ct + 1) * 512],
                        yps[:, ct * 512 : (ct + 1) * 512],
                        pb_sb[:, ct : ct + 1],
                    )
                    nc.sync.dma_start(
                        y_d[ct, :, b * N + nc2 * 512 : b * N + nc2 * 512 + 512],
                        yt[:, ct * 512 : (ct + 1) * 512],
                    )

            SKEW = 2
            K_PROJ = "base"
            prefetch_slab(0)
            for i, step in enumerate(steps):
                if i % 16 == 8 and i // 16 + 1 < 4:
                    prefetch_slab(i // 16 + 1)
                emit_head(step)
                if i >= SKEW:
                    emit_tail(steps[i - SKEW])
                if K_PROJ == "quarter":
                    # quarter (b, nc2) ready once blocks (nc2, 0/1, b) have
                    # normalized (their last tails at steps 23/31/55/63 + SKEW)
                    if i == 26:
                        emit_proj_quarter(0, 0)
                    elif i == 34:
                        emit_proj_quarter(1, 0)
                    elif i == 58:
                        emit_proj_quarter(0, 1)
                else:
                    if i == 58:
                        emit_proj(0, 0)
                    elif i == 61:
                        emit_proj(1, 0)
            for j in range(SKEW, 0, -1):
                emit_tail(steps[len(steps) - j])
            if K_PROJ == "quarter":
                emit_proj_quarter(1, 1)
            else:
                emit_proj(0, 1)
                emit_proj(1, 1)


@functools.cache
def _build_nc():
    nc = bacc.Bacc("TRN2", target_bir_lowering=False, debug=False)
    with tile.TileContext(nc) as tc:
        _emit(tc)
    nc.compile()
    return nc


def _prep_shared(qkv_w, proj_w, proj_b, bias_table, rel_pos_index):
    w2 = np.asarray(qkv_w, np.float32).copy()
    w2[:C] *= SCALE  # fold the attention scale into the q projection
    wqkvT = np.ascontiguousarray(
        w2.T.reshape(2, 128, 3 * C).transpose(1, 0, 2)
    ).astype(NPBF)
    pwT = np.ascontiguousarray(
        np.asarray(proj_w, np.float32).T.reshape(2, 128, C).transpose(1, 0, 2)
    ).astype(NPBF)
    pb = np.ascontiguousarray(
        np.asarray(proj_b, np.float32).reshape(2, 128, 1)
    )
    Braw = np.asarray(bias_table, np.float32)[np.asarray(rel_pos_index)]
    # [n, m, 8] -> Bt[hg, nc2, mp, mt, hl, nn]
    Br = Braw.reshape(2, 512, 8, 128, 2, 4)  # [nc2, nn, mt, mp, hg, hl]
    Bt = Br.transpose(4, 0, 3, 2, 5, 1)
    expb = np.empty((2, 2, 128, 8, 4, 512), np.int16)
    Bc = 127 * 2**7 + SCH_C
    for nc2 in range(2):
        for hg in range(2):
            for mt in range(8):
                chunk = Bt[hg, nc2, :, mt, :, :]  # [mp, hl, nn]
                if TYPE_D[(nc2, hg, mt)]:
                    v = np.rint(SCH_A * chunk + Bc)
                    expb[hg, nc2, :, mt] = np.clip(v, -32768, 32767).astype(
                        np.int16
                    )
                else:
                    expb[hg, nc2, :, mt] = (
                        np.exp(chunk).astype(NPBF).view(np.int16)
                    )
    expb = np.ascontiguousarray(expb.reshape(2, 2, 128, 8, 2048))
    return wqkvT, pwT, pb, expb


def _prep_x(x, c):
    xs = np.asarray(x[c * BPC : (c + 1) * BPC], np.float32)  # [2, 1024, 256]
    a = xs.transpose(2, 0, 1).reshape(C, NT)  # [c_in, b*1024+t]
    return np.ascontiguousarray(a.reshape(2, 128, NT).transpose(1, 0, 2)).astype(NPBF)


def _in_maps(x, qkv_w, proj_w, proj_b, bias_table, rel_pos_index):
    wqkvT, pwT, pb, expb = _prep_shared(
        qkv_w, proj_w, proj_b, bias_table, rel_pos_index
    )
    return [
        {
            "xT": _prep_x(x, c),
            "wqkvT": wqkvT,
            "projwT": pwT,
            "pbias": pb,
            "expb": expb,
        }
        for c in range(NCORES)
    ]


def run(x, qkv_w, proj_w, proj_b, bias_table, rel_pos_index, **run_kwargs):
    nc = _build_nc()
    in_maps = _in_maps(x, qkv_w, proj_w, proj_b, bias_table, rel_pos_index)
    res = run_bass_kernel_spmd(nc, in_maps, list(range(NCORES)), **run_kwargs)
    y = np.stack(
        [
            res.results[c]["yT"].reshape(C, NT).reshape(C, BPC, N).transpose(1, 2, 0)
            for c in range(NCORES)
        ]
    )
    return y.reshape(B, N, C).astype(np.float32), res


def kernel(x, qkv_w, proj_w, proj_b, bias_table, rel_pos_index):
    y, _ = run(x, qkv_w, proj_w, proj_b, bias_table, rel_pos_index)
    return y



# revision 26
# speedup vs baseline: 1.2635x; 1.1344x over previous
"""
Trainium2 Bass kernel for windowed multi-head attention with relative position
bias (Swin-style), data-parallel over the 16 windows across 8 NeuronCores.

Reference computation (per window b of 16, N=1024 tokens, C=256 channels,
H=8 heads, hd=32):
    qkv  = x @ qkv_w.T                    -> q, k, v  [B, H, N, hd]
    attn = softmax(q k^T * hd^-0.5 + bias_table[rel_pos_index])
    out  = (attn @ v)  reshaped -> [B, N, C]
    y    = out @ proj_w.T + proj_b

Device strategy (per core: 2 windows, everything in "transposed" space):
  - host: transpose x -> xT [c, n], pre-scale q-weights by hd^-0.5, gather
    exp(bias_table)[rel_pos_index] to bf16 (softmax is factorized as
    exp(s)*exp(bias), valid since scores are bounded ~|s|<12).
  - qT/kT co-tiles via wqkvT.T @ xT (head h at partitions [32h, 32h+32),
    the layout needed for 4-way tile_position packing of K=32 score
    matmuls); v computed directly in [token, dim] layout via xT.T @ wvT
    (no on-chip transposes anywhere).
  - scoresT[m,n] via row-tiled matmuls; ACT exp(scoresT) PSUM->SBUF bf16
    is the throughput wall (16.8M exps/core, ~1.1us per FD-1024 call);
    DVE does one bf16 2x multiply by the gathered exp-bias per tile.
  - exp offload: the g=1 score tile of even-mt steps (32 of 128 tiles,
    25% of exp work) is drained by the DVE instead: one fused
    scalar_tensor_tensor (scores*A + slab_i16 -> int16, A=2^7/ln2,
    slab = rint(127*128 - 7.5 + A*bias)) writes Schraudolph bf16-bit
    exp(score+bias) directly; those tiles use a dedicated 1-buffer psum
    ring (psc drops to 2 bufs) so the ScalarE and DVE exp streams drain
    concurrently instead of serializing through one ring. ~1.8% RMS
    weight error on offloaded tiles; end-to-end rel_err 0.0071 vs the
    2e-2 budget. Slab regions for offloaded tiles carry the int16
    affine constants; others carry bf16 exp(bias) (bitcast per use).
  - attn@v and the softmax denominator via col-tiled matmuls (4 heads /
    4 ones-stationaries whose M=32 broadcasts each colsum across its
    32-partition group), accumulating over m in PSUM.
  - Software-pipelined emission (scores+exp of step i before mul/av/cs of
    step i-2) keeps the per-engine FIFOs from putting accumulation work in
    the exp ring; PSUM: 3-slot score ring (6 banks) + av + cs (1 each).
  - normalize with reciprocal_approx_fast; projection computed transposed
    (yT = pwT.T @ out_catT, N=512 matmuls) and untransposed on host.
"""

import functools

import ml_dtypes
import numpy as np

import concourse.bacc as bacc
import concourse.tile as tile
from concourse import mybir
from concourse.bass_utils import run_bass_kernel_spmd

BF = mybir.dt.bfloat16
F32 = mybir.dt.float32
NPBF = ml_dtypes.bfloat16

NCORES = 8
B = 16  # windows total
BPC = B // NCORES  # windows per core (2)
N = 1024  # tokens per window
C = 256  # channels
H = 8  # heads
HD = 32  # head dim
SCALE = HD**-0.5
NT = BPC * N  # tokens per core (2048)
EXPF = mybir.ActivationFunctionType.Exp
I16 = mybir.dt.int16

import os
# Schraudolph exp in bf16-bit-space: bits = rint(x*SCH_A + 127*128 + SCH_C)
SCH_A = 2.0**7 / np.log(2.0)
SCH_C = -7.5
# D g-slots: exp of (mt, g=1) tiles in these mt positions runs on the DVE
# (Schraudolph) through a dedicated 1-buffer psum ring, overlapping the
# ScalarE exp stream. Empty = pure baseline.
_dg = os.environ.get("K_DG", "0,2,4,6")
D_MTS = tuple(int(x) for x in _dg.split(",") if x != "")
_dg0 = os.environ.get("K_DG0", "")
D_MTS0 = tuple(int(x) for x in _dg0.split(",") if x != "")

def _is_d(mt, g):
    return (mt in D_MTS) if g == 1 else (mt in D_MTS0)


def _emit(tc):
    nc = tc.nc
    xT_d = nc.dram_tensor("xT", [128, 2, NT], BF, kind="ExternalInput")
    wq_d = nc.dram_tensor("wqkvT", [128, 2, 3 * C], BF, kind="ExternalInput")
    pw_d = nc.dram_tensor("projwT", [128, 2, C], BF, kind="ExternalInput")
    pb_d = nc.dram_tensor("pbias", [2, 128, 1], F32, kind="ExternalInput")
    eb_d = nc.dram_tensor("expb", [2, 2, 128, 8, 2048], I16, kind="ExternalInput")
    y_d = nc.dram_tensor("yT", [2, 128, NT], F32, kind="ExternalOutput")

    with (
        tc.tile_pool(name="const", bufs=1) as cp,
        tc.tile_pool(name="xp", bufs=1) as xp,
        tc.tile_pool(name="qkvp", bufs=1) as qkvp,
        tc.tile_pool(name="vp", bufs=1) as vp,
        tc.tile_pool(name="ebp", bufs=2) as ebp,
        tc.tile_pool(name="esp", bufs=6) as esp,
        tc.tile_pool(name="eap", bufs=4) as eap,
        tc.tile_pool(name="ocp", bufs=1) as ocp,
        tc.tile_pool(name="rcp", bufs=2) as rcp,
        tc.tile_pool(name="yp", bufs=3) as ysp,
    ):
        wq_sb = cp.tile([128, 2, 3 * C], BF)
        pw_sb = cp.tile([128, 2, C], BF)
        pb_sb = cp.tile([128, 2], F32)
        ones128 = cp.tile([128, 32], BF)
        xT_sb = xp.tile([128, 2, NT], BF)
        # qkv co-tiles: 0,1 = q heads 0-3/4-7 ; 2,3 = k ; 4,5 = v
        qkv_sb = qkvp.tile([128, 6, NT], BF)
        # v_aug blocks: [m % 128, b*8+mt, hg*128 + hl*32 + d]
        v_sb = vp.tile([128, 16, 256], BF)
        # out_catT: [co % 128, hg, n]  (co = (4*hg+hl)*32+d, n = b*1024+t)
        oc_sb = ocp.tile([128, 2, NT], BF)

        nc.sync.dma_start(wq_sb[:], wq_d[:])
        if False:  # xT DMA chunking: measured neutral
            for kc in range(2):
                for h in range(2):
                    nc.sync.dma_start(
                        xT_sb[:, kc, h * 1024 : (h + 1) * 1024],
                        xT_d[:, kc, h * 1024 : (h + 1) * 1024],
                    )
        else:
            for kc in range(2):
                nc.sync.dma_start(xT_sb[:, kc, :], xT_d[:, kc, :])
        nc.sync.dma_start(pw_sb[:], pw_d[:])
        nc.sync.dma_start(pb_sb[:], pb_d.rearrange("ct p one -> p (ct one)"))
        nc.gpsimd.memset(ones128[:], 1.0)

        # ---- phase 1: qT/kT co-tiles via wqkvT.T @ xT; v directly in
        # [m, d] layout via xT.T @ wvT (no transposes needed). Evacuations
        # alternate DVE / ScalarE (ScalarE is idle before the exp stream).
        with (
            tc.tile_pool(name="p1", bufs=2, space="PSUM") as p1,
            tc.tile_pool(name="pv", bufs=4, space="PSUM") as pv,
        ):
            def qk_tile(ct, nch2, eng):
                pq = p1.tile([128, 1024], F32, tag="p1", name=f"pq{ct}{nch2}")
                for half in range(2):
                    for kc in range(2):
                        nc.tensor.matmul(
                            pq[:, half * 512 : (half + 1) * 512],
                            wq_sb[:, kc, ct * 128 : (ct + 1) * 128],
                            xT_sb[
                                :,
                                kc,
                                nch2 * 1024
                                + half * 512 : nch2 * 1024
                                + (half + 1) * 512,
                            ],
                            start=(kc == 0),
                            stop=(kc == 1),
                        )
                if eng == 0:
                    nc.vector.tensor_copy(
                        qkv_sb[:, ct, nch2 * 1024 : (nch2 + 1) * 1024], pq[:]
                    )
                else:
                    nc.scalar.copy(
                        qkv_sb[:, ct, nch2 * 1024 : (nch2 + 1) * 1024], pq[:]
                    )

            def v_tile(i, eng):
                pvt = pv.tile([128, 256], F32, tag="pv", name=f"pv{i}")
                for kc in range(2):
                    nc.tensor.matmul(
                        pvt[:],
                        xT_sb[:, kc, i * 128 : (i + 1) * 128],
                        wq_sb[:, kc, 2 * C : 3 * C],
                        start=(kc == 0),
                        stop=(kc == 1),
                    )
                if eng == 0:
                    nc.vector.tensor_copy(v_sb[:, i, :], pvt[:])
                else:
                    nc.scalar.copy(v_sb[:, i, :], pvt[:])

            K_EVAC = "alt"
            def _eng(i):
                return (i % 2) if K_EVAC == "alt" else 0
            e = 0
            if False:  # early qk order: measured neutral
                for ct, nch2 in ((2, 0), (0, 0), (2, 1), (0, 1)):
                    qk_tile(ct, nch2, _eng(e)); e += 1
                for i in range(16):
                    v_tile(i, _eng(i))
                for ct, nch2 in ((3, 0), (1, 0), (3, 1), (1, 1)):
                    qk_tile(ct, nch2, _eng(e)); e += 1
            else:
                for ct in (0, 2):
                    for nch2 in range(2):
                        qk_tile(ct, nch2, _eng(e))
                        e += 1
                for i in range(16):
                    v_tile(i, _eng(i))
                for ct in (1, 3):
                    for nch2 in range(2):
                        qk_tile(ct, nch2, _eng(e))
                        e += 1

        # ---- phase 2: attention, software-pipelined ----
        # Per step (nc2, hg, mt, b): emit the scores matmuls + exps FIRST, then
        # the previous step's bias-mul / attn@v / colsum. This keeps next-step
        # scores ahead of av/cs in the PE FIFO so the ACT engine's ring
        # (exp -> scores -> exp) never includes the accumulation matmuls.
        with (
            tc.tile_pool(
                name="psc", bufs=(2 if (D_MTS or D_MTS0) else 3), space="PSUM"
            ) as psc,
            tc.tile_pool(name="psd", bufs=1, space="PSUM") as psd,
            tc.tile_pool(name="pav", bufs=1, space="PSUM") as pav,
            tc.tile_pool(name="pcs", bufs=1, space="PSUM") as pcs,
        ):
            steps = [
                (nc2, hg, mt, b)
                for nc2 in range(2)
                for hg in range(2)
                for b in range(2)
                for mt in range(8)
            ]
            blocks = {}  # (nc2, hg) -> dict(cs, avs, rc, slab)
            state = {}  # step -> (es, ea)

            def emit_head(step):
                nc2, hg, mt, b = step
                if (nc2, hg, b) not in blocks:
                    cs = pcs.tile([128, 512], F32, tag="cs", name=f"cs{nc2}{hg}{b}")
                    av = pav.tile([128, 512], F32, tag="av", name=f"av{nc2}{hg}{b}")
                    rc = rcp.tile([128, 512], F32, tag="rc", name=f"rc{nc2}{hg}{b}")
                    blocks[(nc2, hg, b)] = dict(
                        slab=slabs[(nc2, hg)], cs=cs, av=av, rc=rc
                    )
                es = esp.tile([128, 2048], BF, tag="es", name=f"es{mt}{b}")
                for g in range(2):
                    if _is_d(mt, g):
                        scp = psd.tile([128, 1024], F32, tag="sd", name="sd")
                    else:
                        scp = psc.tile([128, 1024], F32, tag="sc", name=f"sc{g}")
                    for j in range(2):
                        hl = 2 * g + j
                        nc.tensor.matmul(
                            scp[:, j * 512 : (j + 1) * 512],
                            qkv_sb[
                                32 * hl : 32 * hl + 32,
                                2 + hg,
                                b * N + mt * 128 : b * N + mt * 128 + 128,
                            ],
                            qkv_sb[
                                32 * hl : 32 * hl + 32,
                                hg,
                                b * N + nc2 * 512 : b * N + nc2 * 512 + 512,
                            ],
                            start=True,
                            stop=True,
                            tile_position=(32 * hl, 0),
                        )
                    if _is_d(mt, g):
                        nc.vector.scalar_tensor_tensor(
                            es[:, g * 1024 : (g + 1) * 1024].bitcast(I16),
                            scp[:],
                            float(SCH_A),
                            slabs[(nc2, hg)][:, mt, g * 1024 : (g + 1) * 1024],
                            mybir.AluOpType.mult,
                            mybir.AluOpType.add,
                        )
                    else:
                        nc.scalar.activation(
                            es[:, g * 1024 : (g + 1) * 1024], scp[:], EXPF
                        )
                state[step] = es

            def emit_tail(step):
                nc2, hg, mt, b = step
                blk = blocks[(nc2, hg, b)]
                es = state.pop(step)
                ea = eap.tile([128, 2048], BF)
                d0, d1 = _is_d(mt, 0), _is_d(mt, 1)
                if not (d0 or d1):
                    nc.vector.tensor_mul(
                        ea[:], es[:], blk["slab"][:, mt, :].bitcast(BF)
                    )
                else:
                    # D halves (schraudolph, bias inside) are consumed straight
                    # from es below; only M halves get the exp(bias) multiply.
                    for g in range(2):
                        if not _is_d(mt, g):
                            nc.vector.tensor_mul(
                                ea[:, g * 1024 : (g + 1) * 1024],
                                es[:, g * 1024 : (g + 1) * 1024],
                                blk["slab"][:, mt, g * 1024 : (g + 1) * 1024].bitcast(BF),
                            )
                for hl in range(4):
                    src_t = es if _is_d(mt, hl // 2) else ea
                    nc.tensor.matmul(
                        blk["av"][32 * hl : 32 * hl + 32, :],
                        v_sb[:, b * 8 + mt, hg * 128 + 32 * hl : hg * 128 + 32 * hl + 32],
                        src_t[:, hl * 512 : (hl + 1) * 512],
                        start=(mt == 0),
                        stop=(mt == 7),
                        tile_position=(0, 32 * hl),
                        skip_group_check=True,
                    )
                    nc.tensor.matmul(
                        blk["cs"][32 * hl : 32 * hl + 32, :],
                        ones128[:],
                        src_t[:, hl * 512 : (hl + 1) * 512],
                        start=(mt == 0),
                        stop=(mt == 7),
                        tile_position=(0, 32 * hl),
                        skip_group_check=True,
                    )
                if mt == 7:
                    # this window's colsum is complete: reciprocal + normalize
                    nc.vector.reciprocal_approx_fast(out=blk["rc"][:], in_=blk["cs"][:])
                    nc.vector.tensor_mul(
                        oc_sb[:, hg, b * N + nc2 * 512 : b * N + nc2 * 512 + 512],
                        blk["av"][:],
                        blk["rc"][:],
                    )

            slabs = {}

            def prefetch_slab(bi):
                nc2, hg = [(n, h) for n in range(2) for h in range(2)][bi]
                slab = ebp.tile([128, 8, 2048], I16, tag="slab", name=f"slab{nc2}{hg}")
                nc.sync.dma_start(slab[:], eb_d[hg, nc2])
                slabs[(nc2, hg)] = slab

            def emit_proj(ct, nch):
                yps = psc.tile([128, 1024], F32, tag="sc", name=f"yps{ct}{nch}")
                for half in range(2):
                    for hg in range(2):
                        nc.tensor.matmul(
                            yps[:, half * 512 : (half + 1) * 512],
                            pw_sb[:, hg, ct * 128 : (ct + 1) * 128],
                            oc_sb[
                                :,
                                hg,
                                nch * 1024 + half * 512 : nch * 1024 + (half + 1) * 512,
                            ],
                            start=(hg == 0),
                            stop=(hg == 1),
                            skip_group_check=True,
                        )
                yt = ysp.tile([128, 1024], F32, tag="yt", name=f"yt{ct}{nch}")
                nc.vector.tensor_scalar_add(yt[:], yps[:], pb_sb[:, ct : ct + 1])
                nc.sync.dma_start(y_d[ct, :, nch * 1024 : (nch + 1) * 1024], yt[:])

            def emit_proj_quarter(b, nc2):
                # yT quarter for (window b, half nc2): contract oc over both
                # hg chunks; borrow one sc-ring slot for both ct chunks.
                yps = psc.tile([128, 1024], F32, tag="sc", name=f"ypq{b}{nc2}")
                for ct in range(2):
                    for hg in range(2):
                        nc.tensor.matmul(
                            yps[:, ct * 512 : (ct + 1) * 512],
                            pw_sb[:, hg, ct * 128 : (ct + 1) * 128],
                            oc_sb[:, hg, b * N + nc2 * 512 : b * N + nc2 * 512 + 512],
                            start=(hg == 0),
                            stop=(hg == 1),
                            skip_group_check=True,
                        )
                yt = ysp.tile([128, 1024], F32, tag="yt", name=f"ytq{b}{nc2}")
                for ct in range(2):
                    nc.vector.tensor_scalar_add(
                        yt[:, ct * 512 : (ct + 1) * 512],
                        yps[:, ct * 512 : (ct + 1) * 512],
                        pb_sb[:, ct : ct + 1],
                    )
                    nc.sync.dma_start(
                        y_d[ct, :, b * N + nc2 * 512 : b * N + nc2 * 512 + 512],
                        yt[:, ct * 512 : (ct + 1) * 512],
                    )

            SKEW = 2
            K_PROJ = "base"
            prefetch_slab(0)
            for i, step in enumerate(steps):
                if i % 16 == 8 and i // 16 + 1 < 4:
                    prefetch_slab(i // 16 + 1)
                emit_head(step)
                if i >= SKEW:
                    emit_tail(steps[i - SKEW])
                if K_PROJ == "quarter":
                    # quarter (b, nc2) ready once blocks (nc2, 0/1, b) have
                    # normalized (their last tails at steps 23/31/55/63 + SKEW)
                    if i == 26:
                        emit_proj_quarter(0, 0)
                    elif i == 34:
                        emit_proj_quarter(1, 0)
                    elif i == 58:
                        emit_proj_quarter(0, 1)
                else:
                    if i == 58:
                        emit_proj(0, 0)
                    elif i == 61:
                        emit_proj(1, 0)
            for j in range(SKEW, 0, -1):
                emit_tail(steps[len(steps) - j])
            if K_PROJ == "quarter":
                emit_proj_quarter(1, 1)
            else:
                emit_proj(0, 1)
                emit_proj(1, 1)


@functools.cache
def _build_nc():
    nc = bacc.Bacc("TRN2", target_bir_lowering=False, debug=False)
    with tile.TileContext(nc) as tc:
        _emit(tc)
    nc.compile()
    return nc


def _prep_shared(qkv_w, proj_w, proj_b, bias_table, rel_pos_index):
    w2 = np.asarray(qkv_w, np.float32).copy()
    w2[:C] *= SCALE  # fold the attention scale into the q projection
    wqkvT = np.ascontiguousarray(
        w2.T.reshape(2, 128, 3 * C).transpose(1, 0, 2)
    ).astype(NPBF)
    pwT = np.ascontiguousarray(
        np.asarray(proj_w, np.float32).T.reshape(2, 128, C).transpose(1, 0, 2)
    ).astype(NPBF)
    pb = np.ascontiguousarray(
        np.asarray(proj_b, np.float32).reshape(2, 128, 1)
    )
    Braw = np.asarray(bias_table, np.float32)[np.asarray(rel_pos_index)]
    # [n, m, 8] -> Bt[hg, nc2, mp, mt, hl, nn]
    Br = Braw.reshape(2, 512, 8, 128, 2, 4)  # [nc2, nn, mt, mp, hg, hl]
    Bt = Br.transpose(4, 0, 3, 2, 5, 1)
    expb = np.empty((2, 2, 128, 8, 4, 512), np.int16)
    Bc = 127 * 2**7 + SCH_C
    for nc2 in range(2):
        for hg in range(2):
            for mt in range(8):
                for hl in range(4):
                    chunk = Bt[hg, nc2, :, mt, hl, :]  # [mp, nn]
                    if _is_d(mt, hl // 2):
                        v = np.rint(SCH_A * chunk + Bc)
                        expb[hg, nc2, :, mt, hl] = np.clip(
                            v, -32768, 32767
                        ).astype(np.int16)
                    else:
                        expb[hg, nc2, :, mt, hl] = (
                            np.exp(chunk).astype(NPBF).view(np.int16)
                        )
    expb = np.ascontiguousarray(expb.reshape(2, 2, 128, 8, 2048))
    return wqkvT, pwT, pb, expb


def _prep_x(x, c):
    xs = np.asarray(x[c * BPC : (c + 1) * BPC], np.float32)  # [2, 1024, 256]
    a = xs.transpose(2, 0, 1).reshape(C, NT)  # [c_in, b*1024+t]
    return np.ascontiguousarray(a.reshape(2, 128, NT).transpose(1, 0, 2)).astype(NPBF)


def _in_maps(x, qkv_w, proj_w, proj_b, bias_table, rel_pos_index):
    wqkvT, pwT, pb, expb = _prep_shared(
        qkv_w, proj_w, proj_b, bias_table, rel_pos_index
    )
    return [
        {
            "xT": _prep_x(x, c),
            "wqkvT": wqkvT,
            "projwT": pwT,
            "pbias": pb,
            "expb": expb,
        }
        for c in range(NCORES)
    ]


def run(x, qkv_w, proj_w, proj_b, bias_table, rel_pos_index, **run_kwargs):
    nc = _build_nc()
    in_maps = _in_maps(x, qkv_w, proj_w, proj_b, bias_table, rel_pos_index)
    res = run_bass_kernel_spmd(nc, in_maps, list(range(NCORES)), **run_kwargs)
    y = np.stack(
        [
            res.results[c]["yT"].reshape(C, NT).reshape(C, BPC, N).transpose(1, 2, 0)
            for c in range(NCORES)
        ]
    )
    return y.reshape(B, N, C).astype(np.float32), res


def kernel(x, qkv_w, proj_w, proj_b, bias_table, rel_pos_index):
    y, _ = run(x, qkv_w, proj_w, proj_b, bias_table, rel_pos_index)
    return y

